# revision 1
# baseline (speedup 1.0000x reference)
"""Trainium2 Bass kernel for 2D erosion (3x3 sliding-window min) on
x: (8, 4, 1024, 1024) f32, padded with +1e9 at the borders (pad never wins).

Strategy: pure data parallel over the 32 (b, c) images -> 4 images per core.
The per-core DRAM input is laid out with one 1e9 pad row between/around
images (shape (4*(1024+1)+1, 1024)) so every halo access is affine.

Per image, one SBUF tile [128 partitions x 8192]: partition p holds image
rows 8p..8p+7 concatenated along the free dim. The separable 3-tap min runs
as free-dim-shifted tensor_tensor(min) ops:
  - H(vertical) pass (DVE): row-pair sharing s[k]=min(x[2k],x[2k+1]) then
    combine; boundary rows use a [128, 2048] halo tile holding DRAM rows
    8p-1 and 8p+8 relative to the image (pad rows give border semantics).
  - W(horizontal) pass: pair sharing sw[j]=min(v[2j],v[2j+1]) on DVE, the
    even/odd combines on GPSIMD (idle otherwise; balances the two engines),
    plus tiny strided DVE copies fixing each image row's first/last column.
Output is written in-place into the input tile and DMA'd out on the ACT
HWDGE ring (loads go on the SP ring, so they don't queue behind stores).
"""

import numpy as np

import concourse.bass as bass
import concourse.bacc as bacc
import concourse.mybir as mybir
from concourse.tile import TileContext
from concourse.bass_utils import run_bass_kernel_spmd

N_CORES = 8
B, C, H, W = 8, 4, 1024, 1024
IMGS = B * C // N_CORES  # images per core = 4
P = 128                  # SBUF partitions
R = H // P               # image rows per partition = 8
F = R * W                # free-dim elements per partition = 8192
PAD = 1.0e9
XROWS = IMGS * (H + 1) + 1  # padded per-core input rows
FP32 = mybir.dt.float32
MIN = mybir.AluOpType.min

_NC_CACHE = {}


def _build_nc(reps=1):
    nc = bacc.Bacc()
    x = nc.dram_tensor("x", (XROWS, W), FP32, kind="ExternalInput")
    y = nc.dram_tensor("y", (IMGS, H, W), FP32, kind="ExternalOutput")

    with TileContext(nc) as tc:
        with (
            tc.tile_pool(name="xp", bufs=3) as xpool,
            tc.tile_pool(name="hp", bufs=2) as hpool,
            tc.tile_pool(name="sp", bufs=1) as spool,
            tc.tile_pool(name="vp", bufs=1) as vpool,
            tc.tile_pool(name="wp", bufs=1) as wpool,
        ):
            for i in [im for _ in range(reps) for im in range(IMGS)]:
                base = 1 + i * (H + 1)  # first row of image i in padded DRAM

                xt = xpool.tile([P, F], FP32)
                halo = hpool.tile([P, 2 * W], FP32)

                # main load: image rows are contiguous in DRAM
                xm = x[base : base + H, :].rearrange("(p r) w -> p (r w)", p=P)
                nc.sync.dma_start(out=xt, in_=xm)
                # halo load: partition p gets DRAM rows base-1+8p and base+8+8p
                # (9 rows apart); p=0 low / p=127 high land on 1e9 pad rows.
                hsrc = bass.AP(x, (base - 1) * W, [[R * W, P], [9 * W, 2], [1, W]])
                hdst = halo.rearrange("p (s w) -> p s w", s=2)
                nc.sync.dma_start(out=hdst, in_=hsrc)

                xr = xt.rearrange("p (r w) -> p r w", r=R)
                s = spool.tile([P, (R // 2) * W], FP32)        # [128, 4096]
                sr = s.rearrange("p (r w) -> p r w", r=R // 2)
                v = vpool.tile([P, F], FP32)                   # vertical-min result
                vr = v.rearrange("p (r w) -> p r w", r=R)

                # ---- H pass (DVE): v[r] = min(x[r-1], x[r], x[r+1]) ----
                nc.vector.tensor_tensor(
                    out=sr, in0=xr[:, 0:R:2, :], in1=xr[:, 1:R:2, :], op=MIN
                )
                nc.vector.tensor_tensor(
                    out=vr[:, 2:R:2, :],
                    in0=xr[:, 1 : R - 1 : 2, :],
                    in1=sr[:, 1 : R // 2, :],
                    op=MIN,
                )
                nc.vector.tensor_tensor(
                    out=vr[:, 1 : R - 1 : 2, :],
                    in0=sr[:, 0 : R // 2 - 1, :],
                    in1=xr[:, 2:R:2, :],
                    op=MIN,
                )
                # boundary rows {0, R-1} in one op: halo is one tile/one DMA
                nc.vector.tensor_tensor(
                    out=vr[:, 0 : R : R - 1, :],
                    in0=halo.rearrange("p (s w) -> p s w", s=2),
                    in1=sr[:, 0 : R // 2 : R // 2 - 1, :],
                    op=MIN,
                )

                # ---- W pass: o[j] = min(v[j-1], v[j], v[j+1]) within rows ----
                sw = wpool.tile([P, F // 2], FP32)             # [128, 4096]
                nc.vector.tensor_tensor(
                    out=sw, in0=v[:, 0:F:2], in1=v[:, 1:F:2], op=MIN
                )
                # even cols j=2..8190: min(v[j-1], sw[j/2]); odd j=1..8189
                nc.vector.tensor_tensor(
                    out=xt[:, 2:F:2],
                    in0=v[:, 1 : F - 2 : 2],
                    in1=sw[:, 1 : F // 2],
                    op=MIN,
                )
                nc.vector.tensor_tensor(
                    out=xt[:, 1 : F - 1 : 2],
                    in0=sw[:, 0 : F // 2 - 1],
                    in1=v[:, 2:F:2],
                    op=MIN,
                )
                # per-row first/last column: window shrinks to 2 taps = sw value
                xtr = xt.rearrange("p (r w) -> p r w", r=R)
                swr = sw.rearrange("p (r w) -> p r w", r=R)    # rows of 512
                nc.vector.tensor_copy(out=xtr[:, :, 0:1], in_=swr[:, :, 0:1])
                nc.vector.tensor_copy(
                    out=xtr[:, :, W - 1 : W], in_=swr[:, :, W // 2 - 1 : W // 2]
                )

                # store on the ACT HWDGE ring (parallel to SP loads)
                ym = y[i].rearrange("(p r) w -> p (r w)", p=P)
                nc.scalar.dma_start(out=ym, in_=xt)

    nc.finalize()
    return nc


def _get_nc(reps=1):
    if reps not in _NC_CACHE:
        _NC_CACHE[reps] = _build_nc(reps)
    return _NC_CACHE[reps]


def _pad_shard(shard):
    """(IMGS, H, W) -> (XROWS, W) with a 1e9 pad row between/around images."""
    out = np.full((XROWS, W), PAD, dtype=np.float32)
    for i in range(IMGS):
        base = 1 + i * (H + 1)
        out[base : base + H] = shard[i]
    return out


def kernel(x: np.ndarray, _reps: int = 1):
    x = np.ascontiguousarray(np.asarray(x, dtype=np.float32))
    assert x.shape == (B, C, H, W)
    xs = x.reshape(N_CORES, IMGS, H, W)
    nc = _get_nc(_reps)
    in_maps = [{"x": _pad_shard(xs[k])} for k in range(N_CORES)]
    res = run_bass_kernel_spmd(nc, in_maps, core_ids=list(range(N_CORES)))
    out = np.stack([r["y"] for r in res.results], axis=0)
    return out.reshape(B, C, H, W)



# revision 5
# speedup vs baseline: 6.6965x; 6.6965x over previous
"""Trainium2 Bass kernel for 2D erosion (3x3 sliding-window min) on
x: (8, 4, 1024, 1024) f32. Pure data parallel: 4 images per core.

Layout: host pre-transposes each core's 4 images to partition-major
(128, 4*8*1024) so partition p holds rows 8p..8p+7 of every image as one
contiguous 128KB DRAM block -> ONE load DMA per rep. A second small DRAM
tensor carries the 2 halo rows (8p-1, 8p+8; PAD at image edges) per image.

Compute is a single in-order DVE chain per rep (minimal cross-engine
semaphore hops, which dominate this runtime):
  abuf[k] = min(row k-1, row k)   (bf16, pair tree over rows + halo edges)
  v       = min(abuf, abuf+1row)  (vertical 3-tap done, bf16 2x mode)
  b       = min(v, v>>1)          (bf16 2x)
  out     = min(b, v>>2)          (horizontal 3-tap, f32 out, in-place
                                   into the input tile), then per-row
  col-0 / col-(W-1) fixes from b (2-tap windows at the borders).
One store DMA per rep writes all 4 images back. bf16 scratch keeps SBUF
under budget and doubles DVE throughput; min over bf16-rounded values is
the bf16-rounded min (monotone), so rel err <= 2^-8.
"""

import numpy as np

import concourse.bacc as bacc
import concourse.mybir as mybir
from concourse.tile import TileContext
from concourse.bass_utils import run_bass_kernel_spmd

N_CORES = 8
B, C, H, W = 8, 4, 1024, 1024
IMGS = B * C // N_CORES  # 4 images per core
P = 128                  # SBUF partitions
R = H // P               # 8 image rows per partition
F = R * W                # 8192 elements per image per partition
FT = IMGS * F            # 32768 total main elements per partition
PAD = 1.0e9
FP32 = mybir.dt.float32
BF16 = mybir.dt.bfloat16
MIN = mybir.AluOpType.min

_NC_CACHE = {}


def _build_nc(reps=1):
    nc = bacc.Bacc()
    xm = nc.dram_tensor("xm", (P, FT), FP32, kind="ExternalInput")
    xh = nc.dram_tensor("xh", (P, IMGS * 2 * W), FP32, kind="ExternalInput")
    y = nc.dram_tensor("y", (P, FT), FP32, kind="ExternalOutput")

    with TileContext(nc) as tc:
        with (
            tc.tile_pool(name="xp", bufs=4) as xpool,
            tc.tile_pool(name="hp", bufs=3) as hpool,
            tc.tile_pool(name="ap", bufs=1) as apool,
            tc.tile_pool(name="vp", bufs=1) as vpool,
        ):
            A = apool.tile([P, (R + 1) * W], BF16)  # 9 row-pair mins
            V = vpool.tile([P, F], BF16)            # vertical 3-tap result

            for it in range(reps * IMGS):
                i = it % IMGS
                o = i * F
                ho = i * 2 * W
                Xi = xpool.tile([P, F], FP32)
                HLi = hpool.tile([P, 2 * W], FP32)
                nc.sync.dma_start(out=Xi, in_=xm[:, o : o + F])
                nc.sync.dma_start(out=HLi, in_=xh[:, ho : ho + 2 * W])

                lo = HLi[:, 0:W]          # row 8p-1 (or PAD)
                hi = HLi[:, W : 2 * W]    # row 8p+8 (or PAD)
                # --- vertical pass ---
                # A[k] = min(row k-1, row k), k=0..8 (rows -1..8)
                nc.vector.tensor_tensor(
                    out=A[:, 0:W], in0=lo, in1=Xi[:, 0:W], op=MIN
                )
                nc.vector.tensor_tensor(
                    out=A[:, W : R * W],
                    in0=Xi[:, 0 : (R - 1) * W],
                    in1=Xi[:, W : R * W],
                    op=MIN,
                )
                nc.vector.tensor_tensor(
                    out=A[:, R * W : (R + 1) * W],
                    in0=Xi[:, (R - 1) * W : R * W],
                    in1=hi,
                    op=MIN,
                )
                # V[r] = min(A[r], A[r+1]) = min(rows r-1, r, r+1)
                nc.vector.tensor_tensor(
                    out=V, in0=A[:, 0 : R * W], in1=A[:, W : (R + 1) * W], op=MIN
                )
                # --- horizontal pass (B reuses A's buffer) ---
                Bb = A
                nc.vector.tensor_tensor(
                    out=Bb[:, 0 : F - 1], in0=V[:, 0 : F - 1], in1=V[:, 1:F], op=MIN
                )
                # interior: out[j] = min(B[j-1], V[j+1]), j = 1..F-2
                nc.vector.tensor_tensor(
                    out=Xi[:, 1 : F - 1],
                    in0=Bb[:, 0 : F - 2],
                    in1=V[:, 2:F],
                    op=MIN,
                )
                # per-row border columns: 2-tap windows = B values
                xr = Xi.rearrange("p (q w) -> p q w", w=W)  # q = R rows
                br = Bb.rearrange("p (q w) -> p q w", w=W)  # q = R+1 rows
                nc.vector.tensor_copy(out=xr[:, :, 0:1], in_=br[:, 0:R, 0:1])
                nc.vector.tensor_copy(
                    out=xr[:, :, W - 1 : W], in_=br[:, 0:R, W - 2 : W - 1]
                )

                nc.scalar.dma_start(out=y[:, o : o + F], in_=Xi)

    nc.finalize()
    return nc


def _get_nc(reps=1):
    if reps not in _NC_CACHE:
        _NC_CACHE[reps] = _build_nc(reps)
    return _NC_CACHE[reps]


def _prep_shard(shard):
    """(IMGS, H, W) -> xm (P, FT) partition-major, xh (P, IMGS*2*W) halos."""
    xm = np.ascontiguousarray(
        shard.reshape(IMGS, P, R, W).transpose(1, 0, 2, 3)
    ).reshape(P, FT)
    lo = np.full((P, IMGS, W), PAD, dtype=np.float32)
    hi = np.full((P, IMGS, W), PAD, dtype=np.float32)
    lo[1:] = shard[:, R - 1 : (P - 1) * R : R, :].transpose(1, 0, 2)
    hi[:-1] = shard[:, R : (P - 1) * R + 1 : R, :].transpose(1, 0, 2)
    xh = np.stack([lo, hi], axis=2).reshape(P, IMGS * 2 * W)
    return xm, xh


def kernel(x: np.ndarray, _reps: int = 1):
    x = np.ascontiguousarray(np.asarray(x, dtype=np.float32))
    assert x.shape == (B, C, H, W)
    xs = x.reshape(N_CORES, IMGS, H, W)
    nc = _get_nc(_reps)
    in_maps = []
    for k in range(N_CORES):
        xm, xh = _prep_shard(xs[k])
        in_maps.append({"xm": xm, "xh": xh})
    res = run_bass_kernel_spmd(nc, in_maps, core_ids=list(range(N_CORES)))
    out = np.stack(
        [
            r["y"].reshape(P, IMGS, R, W).transpose(1, 0, 2, 3).reshape(IMGS, H, W)
            for r in res.results
        ],
        axis=0,
    )
    return out.reshape(B, C, H, W)
